# revision 6
# baseline (speedup 1.0000x reference)
"""KSparseFFTClassifier Trainium2 kernel.

Math: reference computes
    h   = x @ W_proj.T + b_proj                      (bs, 129)
    h  *= scale  (sqrt(2) on dims 1..64)
    out = IDFT65(h[:, :65]) + h[:, 65:] @ Ws.T       (bs, 16384)

The zero-padded orthonormal IDFT of the 65 nonzero frequency components is a
dense matmul against a (65, N) cos/sin basis; the DC row of that basis is the
constant 1/sqrt(N).  So with M = [scaled cos/sin basis for h dims 1..64;
Ws.T]  (128 x N):

    out[b, n] = h[b, 1:129] @ M[:, n] + (h[b, 0] + 0) / sqrt(N)

i.e. a (bs,2048)x(2048,128) matmul, a (bs,128)x(128,N) matmul, and a
per-row scalar (the DC term) added during PSUM eviction.

Sharding: data-parallel over batch, 512 rows per core on 8 cores.

The kernel is DMA-bound (23.6 MB/core of fp16 traffic); input loads ride the
Activation HWDGE queue while output stores ride the SP queue so the two
streams overlap.  PSUM eviction (with the DC-term add fused in) rotates over
the Activation, DVE and Pool engines to keep pace with the store stream.
"""

import numpy as np

BS = 4096
IN_DIM = 2048
N = 16384
K = 32
SLACK = 64
NCORES = 8
BC = BS // NCORES        # 512 batch rows per core
P = 128
KT = IN_DIM // P         # 16 contraction tiles for matmul1
NCHUNK = 4096            # output column chunk (SBUF out tile free size)
NCH = N // NCHUNK        # 4

MM1_DT = "float16"
MM2_DT = "float16"

_NC_CACHE = {}


def _build_nc(mm1_name, mm2_name):
    import concourse.bacc as bacc
    import concourse.mybir as mybir
    import concourse.tile as tile

    f32 = mybir.dt.float32
    mm1 = getattr(mybir.dt, mm1_name)
    mm2 = getattr(mybir.dt, mm2_name)

    nc = bacc.Bacc("TRN2", target_bir_lowering=False)

    # wt = [w1t (KT*P cols) | w0 (KT cols)] packed fp16
    wt = nc.dram_tensor("wt", [P, KT * P + KT], mm1, kind="ExternalInput")
    xT = nc.dram_tensor("xT", [P, KT * BC], mm1, kind="ExternalInput")
    mmat = nc.dram_tensor("mmat", [P, N], mm2, kind="ExternalInput")
    # consts f32: col 0 = bt (bias for h dims 1..128), [0,1] = b0/sqrt(N)
    consts = nc.dram_tensor("consts", [P, 2], f32, kind="ExternalInput")
    out = nc.dram_tensor("out", [BC, N], mm2, kind="ExternalOutput")

    Ident = mybir.ActivationFunctionType.Identity

    with tile.TileContext(nc) as tc:
        with (
            tc.tile_pool(name="wp", bufs=1) as wp,
            tc.tile_pool(name="xp", bufs=1) as xp,
            tc.tile_pool(name="mp", bufs=1) as mp,
            tc.tile_pool(name="hp", bufs=1) as hp,
            tc.tile_pool(name="op", bufs=3) as op,
            tc.tile_pool(name="ps", bufs=5, space="PSUM") as ps,
            tc.tile_pool(name="ps1", bufs=1, space="PSUM") as ps1,
            tc.tile_pool(name="ps2", bufs=1, space="PSUM") as ps2,
        ):
            # input loads on the Activation HWDGE queue (outputs use SP)
            wt_sb = wp.tile([P, KT * P + KT], mm1, tag="wt")
            nc.scalar.dma_start(out=wt_sb[:, :], in_=wt[:, :])
            xg = []
            for g in range(4):
                t = xp.tile([P, 4 * BC], mm1, tag=f"xg{g}")
                nc.scalar.dma_start(out=t[:, :], in_=xT[:, g * 4 * BC:(g + 1) * 4 * BC])
                xg.append(t)
            mm = []
            for ti in range(NCH):
                m = mp.tile([P, NCHUNK], mm2, tag=f"m{ti}")
                nc.scalar.dma_start(out=m[:, :], in_=mmat[:, ti * NCHUNK:(ti + 1) * NCHUNK])
                mm.append(m)
            cst_sb = wp.tile([P, 2], f32, tag="cst")
            nc.scalar.dma_start(out=cst_sb[:, :], in_=consts[:, :])
            ones_sb = wp.tile([1, 1], mm1, tag="ones")
            nc.vector.memset(ones_sb[:, :], 1.0)

            # matmul1: hT[d, b] for d = h dims 1..128
            hT_ps = ps1.tile([P, BC], f32, tag="hT")
            for kt in range(KT):
                nc.tensor.matmul(
                    hT_ps[:, :],
                    lhsT=wt_sb[:, kt * P:(kt + 1) * P],
                    rhs=xg[kt // 4][:, (kt % 4) * BC:(kt % 4 + 1) * BC],
                    start=(kt == 0),
                    stop=(kt == KT - 1),
                )
            hT_sb = hp.tile([P, BC], mm2, tag="hT_sb")
            nc.scalar.add(hT_sb[:, :], hT_ps[:, :], cst_sb[:, 0:1])

            # dc row: h dim 0 as (1, BC), on PE while ACT evicts hT
            dcr_ps = ps2.tile([1, BC], f32, tag="dcr")
            for kt in range(KT):
                nc.tensor.matmul(
                    dcr_ps[:, :],
                    lhsT=wt_sb[:, KT * P + kt:KT * P + kt + 1],
                    rhs=xg[kt // 4][:, (kt % 4) * BC:(kt % 4 + 1) * BC],
                    start=(kt == 0),
                    stop=(kt == KT - 1),
                )
            dcr_sb = hp.tile([1, BC], mm1, tag="dcr_sb")
            nc.scalar.activation(
                dcr_sb[:, :], dcr_ps[:, :], Ident,
                bias=cst_sb[0:1, 1:2], scale=float(1.0 / np.sqrt(N)),
            )
            # PE-transpose (1, BC) -> (P, BC//P): 4 col writes into one tile
            dcc_ps = ps2.tile([P, BC // P], f32, tag="dcc")
            for j in range(BC // P):
                nc.tensor.matmul(
                    dcc_ps[:, j:j + 1],
                    lhsT=dcr_sb[0:1, j * P:(j + 1) * P],
                    rhs=ones_sb[0:1, 0:1],
                    start=True,
                    stop=True,
                )
            dc_sb = hp.tile([P, BC // P], f32, tag="dc_sb")
            nc.scalar.copy(dc_sb[:, :], dcc_ps[:, :])

            # matmul2 + DC bias-add eviction (3-engine rotation) + store
            ev = 0
            for ti in range(NCH):
                for j in range(BC // P):
                    ob = op.tile([P, NCHUNK], mm2, tag="ob")
                    for s in range(NCHUNK // 512):
                        pt = ps.tile([P, 512], f32, tag="mm2")
                        nc.tensor.matmul(
                            pt[:, :],
                            lhsT=hT_sb[:, j * P:(j + 1) * P],
                            rhs=mm[ti][:, s * 512:(s + 1) * 512],
                            start=True,
                            stop=True,
                        )
                        dst = ob[:, s * 512:(s + 1) * 512]
                        if ev % 2 == 0:
                            nc.scalar.add(dst, pt[:, :], dc_sb[:, j:j + 1])
                        else:
                            nc.vector.tensor_scalar_add(dst, pt[:, :], dc_sb[:, j:j + 1])
                        ev += 1
                    nc.sync.dma_start(
                        out=out[j * P:(j + 1) * P, ti * NCHUNK:(ti + 1) * NCHUNK],
                        in_=ob[:, :],
                    )
    nc.compile()
    return nc


def _get_nc():
    key = (MM1_DT, MM2_DT)
    if key not in _NC_CACHE:
        _NC_CACHE[key] = _build_nc(*key)
    return _NC_CACHE[key]


def _np_dt(name):
    import ml_dtypes
    return {"float16": np.float16, "bfloat16": ml_dtypes.bfloat16,
            "float32": np.float32, "float32r": np.float32}[name]


def _host_pack(x, W_proj, b_proj, Ws):
    dt1 = _np_dt(MM1_DT)
    dt2 = _np_dt(MM2_DT)
    SQRT2 = np.float64(np.sqrt(np.float32(2.0)))
    n_idx = np.arange(N, dtype=np.float64)
    k_idx = np.arange(1, K + 1, dtype=np.float64)
    theta = (2.0 * np.pi / N) * np.outer(k_idx, n_idx)
    M = np.empty((P, N), np.float32)
    isqn = 1.0 / np.sqrt(np.float64(N))
    M[0:2 * K:2] = (SQRT2 * isqn) * np.cos(theta)
    M[1:2 * K:2] = (SQRT2 * isqn) * np.sin(theta)
    M[2 * K:] = Ws.T
    M = M.astype(dt2)

    w1 = W_proj[1:P + 1]                                  # (128, 2048)
    wt = np.empty((P, KT * P + KT), dt1)
    wt[:, :KT * P] = w1.T.reshape(KT, P, P).transpose(1, 0, 2).reshape(P, KT * P)
    wt[:, KT * P:] = W_proj[0].reshape(KT, P).T            # w0
    consts = np.zeros((P, 2), np.float32)
    consts[:, 0] = b_proj[1:P + 1]
    consts[0, 1] = np.float32(b_proj[0] / np.sqrt(np.float64(N)))

    xts = []
    for c in range(NCORES):
        xc = x[c * BC:(c + 1) * BC]                        # (512, 2048)
        xt = np.ascontiguousarray(
            xc.T.reshape(KT, P, BC).transpose(1, 0, 2).reshape(P, KT * BC)
        ).astype(dt1)
        xts.append(xt)
    return M, wt, consts, xts


def kernel(x, W_proj, b_proj, Ws, _trace=False, _tmpdir=None):
    from concourse import bass_utils

    x = np.ascontiguousarray(x, np.float32)
    W_proj = np.ascontiguousarray(W_proj, np.float32)
    b_proj = np.ascontiguousarray(b_proj, np.float32)
    Ws = np.ascontiguousarray(Ws, np.float32)

    M, wt, consts, xts = _host_pack(x, W_proj, b_proj, Ws)
    nc = _get_nc()

    in_maps = [
        {"xT": xts[c], "wt": wt, "mmat": M, "consts": consts}
        for c in range(NCORES)
    ]
    kw = {}
    if _trace:
        kw = dict(trace=True, tmpdir=_tmpdir, trace_cores=[0])
    res = bass_utils.run_bass_kernel_spmd(nc, in_maps, core_ids=list(range(NCORES)), **kw)
    out = np.concatenate([r["out"] for r in res.results], axis=0).astype(np.float32)
    if _trace:
        return out, res
    return out
